# revision 10
# baseline (speedup 1.0000x reference)
"""Trainium2 Bass kernel for ContrastSense-without-queue (topk_masking).

Strategy (8 NeuronCores, data-parallel over rows of q):
  - Host: stable-sort columns (k rows) by domain so each domain is a
    contiguous column segment; replicate k/time_label to all cores.
  - Device per core (512 q-rows): normalize q/k (fp32, Newton-refined
    rsqrt), compute sims = qn @ kn_perm.T with true-fp32 PE matmuls,
    then find, for every (row, domain), the exact k_d-th-largest
    similarity via a 25-step vectorized bisection (strict counts done
    with fused compare+accumulate ops on the Scalar and Vector
    engines), build the |dt|<50 time mask, and emit
    logits = -inf where masked else sims/temp.
  - Host: scatter columns back to the original order, prepend l_pos.

Outputs match the reference tuple (logits [N,N+1] f32, targets [N] i32,
feature [N,N] f32).
"""

import os
import sys

import numpy as np

for _p in ("/opt/trn_rl_repo", os.path.expanduser("~/.axon_site/_ro/trn_rl_repo")):
    if os.path.isdir(_p) and _p not in sys.path:
        sys.path.insert(0, _p)

import concourse.bass as bass  # noqa: E402
import concourse.mybir as mybir  # noqa: E402
from concourse import bacc  # noqa: E402
from concourse.bass_utils import run_bass_kernel_spmd  # noqa: E402
from concourse.tile import TileContext  # noqa: E402

F32 = mybir.dt.float32
OP = mybir.AluOpType
AF = mybir.ActivationFunctionType

N = 4096
C = 256
ND = 8
NCORES = 8
RPC = N // NCORES   # rows per core = 512
RT = RPC // 128     # row tiles per core = 4
LAST_RATIO = 0.7
TEMP = 0.07
TINV = float(1.0 / np.float32(TEMP))  # logits = v * (1/temp); <=1ulp vs v/temp
NEGBIG = -3.4028235e38  # most-negative normal fp32; doubled -> -inf

NITER_SAFE = 25
WIN_SAFE = (-1.002, 1.002)  # fully data-independent window
NITER_FAST = 21
# thresholds are 30th-pctile order stats of ~N(0, 1/256) samples: concentrated
# around -0.033 +- ~0.004; +-7 sigma window, device-verified with fallback
WIN_FAST = (-0.061, -0.005)

_BUILD_CACHE = {}
LAST_RESULT = None  # BassKernelResults of the most recent run (for test.py)


def _engine_split(counts):
    """Assign each (rowtile, domain) counting stream to ACT or DVE,
    greedily balancing modeled per-iteration cost."""
    act_cost = 0.0
    dve_cost = 4600.0  # DVE also runs updates + apply/evac work
    eng = {}
    pairs = [(r, d) for r in range(RT) for d in range(ND)]
    # big segments first for better greedy balance
    pairs.sort(key=lambda p: -counts[p[1]])
    for r, d in pairs:
        c = float(counts[d])
        a = 429.0 + 0.833 * c  # ACT sign+accum (sim-calibrated)
        v = 60.0 + 0.52 * c    # DVE ts+accum runs 2x (sim-calibrated)
        if act_cost + a <= dve_cost + v:
            eng[(r, d)] = "act"
            act_cost += a
        else:
            eng[(r, d)] = "dve"
            dve_cost += v
    return eng


def _build(counts, prefix, keeps, eng, win, niter):
    nc = bacc.Bacc("TRN2")
    cmax = int(max(int(c) for c in counts if c > 0))

    qs = nc.dram_tensor("qs", [RPC, C], F32, kind="ExternalInput")
    ko = nc.dram_tensor("ko", [RPC, C], F32, kind="ExternalInput")
    kp = nc.dram_tensor("kp", [N, C], F32, kind="ExternalInput")
    tb_d = nc.dram_tensor("tb", [128, N], F32, kind="ExternalInput")
    trlo_d = nc.dram_tensor("trlo", [128, RT], F32, kind="ExternalInput")
    trhi_d = nc.dram_tensor("trhi", [128, RT], F32, kind="ExternalInput")
    thr_d = nc.dram_tensor("thr", [128, RT * ND], F32, kind="ExternalInput")
    id_d = nc.dram_tensor("ident", [128, 128], F32, kind="ExternalInput")

    feat = nc.dram_tensor("feat", [RPC, N], F32, kind="ExternalOutput")
    lneg = nc.dram_tensor("lneg", [RPC, N], F32, kind="ExternalOutput")
    lpos = nc.dram_tensor("lpos", [128, RT], F32, kind="ExternalOutput")
    cchk = nc.dram_tensor("cchk", [128, RT * ND], F32, kind="ExternalOutput")

    NQ = RT          # q tiles
    NK = N // 128    # kp tiles = 32
    NCC = C // 128   # contraction chunks = 2
    NNC = N // 512   # 512-wide column chunks = 8

    with TileContext(nc) as tc:
        with tc.tile_pool(name="persist", bufs=1) as pp, \
             tc.tile_pool(name="vpool", bufs=1) as vp:

            tb = pp.tile([128, N], F32, tag="tb")
            nc.sync.dma_start(tb[:], tb_d[:])
            trlo = pp.tile([128, RT], F32, tag="trlo")
            nc.sync.dma_start(trlo[:], trlo_d[:])
            trhi = pp.tile([128, RT], F32, tag="trhi")
            nc.sync.dma_start(trhi[:], trhi_d[:])
            thr = pp.tile([128, RT * ND], F32, tag="thr")
            nc.sync.dma_start(thr[:], thr_d[:])

            v = [vp.tile([128, N], F32, tag=f"v{r}", name=f"v{r}") for r in range(RT)]

            # ---------------- load + normalize + transpose + matmul ----
            with tc.tile_pool(name="raw", bufs=1) as rp, \
                 tc.tile_pool(name="tpool", bufs=1) as tp, \
                 tc.tile_pool(name="pmm", bufs=4, space="PSUM") as pmm, \
                 tc.tile_pool(name="ptr", bufs=4, space="PSUM") as ptr:

                ident = tp.tile([128, 128], F32, tag="ident")
                nc.sync.dma_start(ident[:], id_d[:])

                nt = NQ + NQ + NK  # 40 norm streams: q, ko, kp
                nrm = pp.tile([128, nt], F32, tag="nrm")
                s_ssq = nrm  # alias names for clarity
                qraw = [rp.tile([128, C], F32, tag=f"q{r}", name=f"q{r}") for r in range(NQ)]
                koraw = [rp.tile([128, C], F32, tag=f"o{r}", name=f"o{r}") for r in range(NQ)]
                kraw = [rp.tile([128, C], F32, tag=f"k{j}", name=f"k{j}") for j in range(NK)]
                sq_scr = [rp.tile([128, C], F32, tag=f"sqs{i}", name=f"sqs{i}") for i in range(3)]

                for r in range(NQ):
                    nc.sync.dma_start(qraw[r][:], qs[r * 128:(r + 1) * 128, :])
                for r in range(NQ):
                    nc.sync.dma_start(koraw[r][:], ko[r * 128:(r + 1) * 128, :])
                for j in range(NK):
                    nc.sync.dma_start(kraw[j][:], kp[j * 128:(j + 1) * 128, :])

                allt = qraw + koraw + kraw
                for i, t in enumerate(allt):
                    nc.scalar.activation(sq_scr[i % 3][:], t[:], AF.Square,
                                         bias=0.0, scale=1.0,
                                         accum_out=s_ssq[:, i:i + 1])
                # rsqrt with one Newton step:  y1 = y0*(1.5 - 0.5*ssq*y0^2)
                sroot = pp.tile([128, nt], F32, tag="sroot")
                nc.scalar.activation(sroot[:], nrm[:], AF.Sqrt, bias=0.0, scale=1.0)
                y0 = pp.tile([128, nt], F32, tag="y0")
                nc.vector.reciprocal(y0[:], sroot[:])
                tmp = pp.tile([128, nt], F32, tag="nrmtmp")
                nc.vector.tensor_tensor(tmp[:], y0[:], y0[:], op=OP.mult)
                nc.vector.tensor_tensor(tmp[:], tmp[:], nrm[:], op=OP.mult)
                nc.vector.tensor_scalar(tmp[:], tmp[:], -0.5, 1.5, op0=OP.mult, op1=OP.add)
                nc.vector.tensor_tensor(y0[:], y0[:], tmp[:], op=OP.mult)

                # normalize in place
                for i, t in enumerate(allt):
                    nc.vector.tensor_scalar(t[:], t[:], y0[:, i:i + 1], None, op0=OP.mult)

                # l_pos = rowsum(qn*kon) * (1/temp)
                lpt = pp.tile([128, RT], F32, tag="lpt")
                for r in range(NQ):
                    nc.vector.tensor_tensor(sq_scr[r % 3][:], qraw[r][:], koraw[r][:], op=OP.mult)
                    nc.vector.tensor_reduce(lpt[:, r:r + 1], sq_scr[r % 3][:],
                                            axis=mybir.AxisListType.X, op=OP.add)
                nc.vector.tensor_scalar(lpt[:], lpt[:], TINV, None, op0=OP.mult)
                nc.sync.dma_start(lpos[:], lpt[:])

                # transposes: qnT [2 x (128 x 512)], knT [2 x (128 x 4096)]
                qnT = [tp.tile([128, RPC], F32, tag=f"qnT{cc}", name=f"qnT{cc}") for cc in range(NCC)]
                knT = [tp.tile([128, N], F32, tag=f"knT{cc}", name=f"knT{cc}") for cc in range(NCC)]
                # evacuate transposes with DVE only: keeps every matmul's
                # dependencies on a single engine clock (fp32 matmuls are
                # self-loading, so multi-sem waits can't be moved to LDW)
                for cc in range(NCC):
                    pt = ptr.tile([128, 512], F32, tag="ptr")
                    for r in range(NQ):
                        nc.tensor.transpose(pt[:, r * 128:(r + 1) * 128],
                                            qraw[r][:, cc * 128:(cc + 1) * 128], ident[:])
                    nc.vector.tensor_copy(qnT[cc][:, :], pt[:])
                for jg in range(NK // 4):
                    for cc in range(NCC):
                        pt = ptr.tile([128, 512], F32, tag="ptr")
                        for jj in range(4):
                            j = jg * 4 + jj
                            nc.tensor.transpose(pt[:, jj * 128:(jj + 1) * 128],
                                                kraw[j][:, cc * 128:(cc + 1) * 128], ident[:])
                        nc.vector.tensor_copy(knT[cc][:, jg * 512:(jg + 1) * 512], pt[:])

                # main matmul: v[r][:, n*512:(n+1)*512] = qnT.T @ knT chunks
                for r in range(RT):
                    for n in range(NNC):
                        pm = pmm.tile([128, 512], F32, tag="pmm")
                        for cc in range(NCC):
                            nc.tensor.matmul(pm[:],
                                             qnT[cc][:, r * 128:(r + 1) * 128],
                                             knT[cc][:, n * 512:(n + 1) * 512],
                                             start=(cc == 0), stop=(cc == NCC - 1))
                        nc.scalar.copy(v[r][:, n * 512:(n + 1) * 512], pm[:])
                        nc.sync.dma_start(
                            feat[r * 128:(r + 1) * 128, n * 512:(n + 1) * 512],
                            v[r][:, n * 512:(n + 1) * 512])

            # ---------------- selection: vectorized bisection ----------
            NP = RT * ND
            lo = pp.tile([128, NP], F32, tag="lo")
            hi = pp.tile([128, NP], F32, tag="hi")
            mid = pp.tile([128, NP], F32, tag="mid")
            nmid = pp.tile([128, NP], F32, tag="nmid")
            cnt = pp.tile([128, NP], F32, tag="cnt")
            dec = pp.tile([128, NP], F32, tag="dec")
            st0 = pp.tile([128, NP], F32, tag="st0")
            su0 = pp.tile([128, NP], F32, tag="su0")
            su1 = pp.tile([128, NP], F32, tag="su1")
            wlo, whi = win
            mid0 = 0.5 * (wlo + whi)
            nc.vector.memset(lo[:], wlo)
            nc.vector.memset(hi[:], whi)
            nc.vector.memset(mid[:], mid0)
            nc.vector.memset(nmid[:], -mid0)

            with tc.tile_pool(name="cscr", bufs=3) as cs:
                for it in range(niter):
                    for r in range(RT):
                        for d in range(ND):
                            cd = int(counts[d])
                            if cd == 0:
                                continue
                            col = r * ND + d
                            s0 = int(prefix[d])
                            seg = v[r][:, s0:s0 + cd]
                            if eng[(r, d)] == "act":
                                o = cs.tile([128, cmax], F32, tag="sa")
                                nc.scalar.activation(o[:, :cd], seg, AF.Sign,
                                                     bias=nmid[:, col:col + 1],
                                                     scale=1.0,
                                                     accum_out=cnt[:, col:col + 1])
                            else:
                                o = cs.tile([128, cmax], F32, tag="sd")
                                nc.vector.tensor_scalar(o[:, :cd], seg,
                                                        mid[:, col:col + 1], None,
                                                        op0=OP.is_gt, op1=OP.add,
                                                        accum_out=cnt[:, col:col + 1])
                    nc.vector.tensor_tensor(dec[:], cnt[:], thr[:], op=OP.is_ge)
                    nc.vector.tensor_tensor(st0[:], mid[:], lo[:], op=OP.subtract)
                    nc.vector.tensor_tensor(st0[:], dec[:], st0[:], op=OP.mult)
                    nc.vector.tensor_tensor(lo[:], lo[:], st0[:], op=OP.add)
                    nc.vector.tensor_tensor(su0[:], mid[:], hi[:], op=OP.subtract)
                    nc.vector.tensor_tensor(su1[:], dec[:], su0[:], op=OP.mult)
                    nc.vector.tensor_tensor(su0[:], su0[:], su1[:], op=OP.subtract)
                    nc.vector.tensor_tensor(hi[:], hi[:], su0[:], op=OP.add)
                    if it < niter - 1:
                        nc.vector.tensor_tensor(mid[:], lo[:], hi[:], op=OP.add)
                        nc.vector.tensor_scalar(mid[:], mid[:], 0.5, None, op0=OP.mult)
                        nc.vector.tensor_scalar(nmid[:], mid[:], -1.0, None, op0=OP.mult)

            # verification: recount at tau = lo; host asserts exact counts
            nlo = pp.tile([128, NP], F32, tag="nlo")
            nc.vector.tensor_scalar(nlo[:], lo[:], -1.0, None, op0=OP.mult)
            with tc.tile_pool(name="vscr", bufs=3) as vs_p:
                for r in range(RT):
                    for d in range(ND):
                        cd = int(counts[d])
                        if cd == 0:
                            continue
                        col = r * ND + d
                        s0 = int(prefix[d])
                        seg = v[r][:, s0:s0 + cd]
                        if eng[(r, d)] == "act":
                            o = vs_p.tile([128, cmax], F32, tag="va", name="va")
                            nc.scalar.activation(o[:, :cd], seg, AF.Sign,
                                                 bias=nlo[:, col:col + 1],
                                                 scale=1.0,
                                                 accum_out=cnt[:, col:col + 1])
                        else:
                            o = vs_p.tile([128, cmax], F32, tag="vd", name="vd")
                            nc.vector.tensor_scalar(o[:, :cd], seg,
                                                    lo[:, col:col + 1], None,
                                                    op0=OP.is_gt, op1=OP.add,
                                                    accum_out=cnt[:, col:col + 1])
            nc.sync.dma_start(cchk[:], cnt[:])

            # ---------------- time mask + apply + store ----------------
            with tc.tile_pool(name="mke", bufs=1) as mk, \
                 tc.tile_pool(name="c2p", bufs=1) as c2p, \
                 tc.tile_pool(name="e2p", bufs=1) as e2p:
                for r in range(RT):
                    # time mask: m = (t_j > t_i - 49.5) && (t_j < t_i + 49.5)
                    c2 = c2p.tile([128, N], F32, tag="c2")
                    nc.gpsimd.tensor_scalar(c2[:], tb[:], trlo[:, r:r + 1], None,
                                            op0=OP.is_gt)
                    m = mk.tile([128, N], F32, tag="m")
                    nc.vector.scalar_tensor_tensor(m[:], tb[:], trhi[:, r:r + 1],
                                                   c2[:], op0=OP.is_lt,
                                                   op1=OP.logical_and)
                    # eliminate = (v <= lo) per domain segment
                    e2 = e2p.tile([128, N], F32, tag="e2")
                    for d in range(ND):
                        cd = int(counts[d])
                        if cd == 0:
                            continue
                        col = r * ND + d
                        s0 = int(prefix[d])
                        nc.vector.tensor_scalar(e2[:, s0:s0 + cd],
                                                v[r][:, s0:s0 + cd],
                                                lo[:, col:col + 1], None,
                                                op0=OP.is_le)
                    nc.vector.tensor_tensor(e2[:], e2[:], m[:], op=OP.logical_or)
                    # vs = v/temp (into c2, dead); t1 = e2*NEGBIG (into m, dead)
                    # logits = t1*2 + vs  (2*NEGBIG overflows to -inf)
                    nc.gpsimd.tensor_scalar(m[:], e2[:], NEGBIG, None, op0=OP.mult)
                    nc.scalar.mul(c2[:], v[r][:], TINV)
                    nc.vector.scalar_tensor_tensor(v[r][:], m[:], 2.0, c2[:],
                                                   op0=OP.mult, op1=OP.add)
                    nc.sync.dma_start(lneg[r * 128:(r + 1) * 128, :], v[r][:])

    nc.finalize()
    return nc


def _verify_counts(outs, counts, keeps, eng):
    for om in outs:
        cc = om["cchk"]
        for r in range(RT):
            for d in range(ND):
                if counts[d] == 0:
                    continue
                col = r * ND + d
                got = cc[:, col]
                if eng[(r, d)] == "act":
                    base = float(2 * keeps[d] - counts[d])
                    ok = np.all((got >= base - 0.1) & (got <= base + 1.1))
                else:
                    ok = np.all(np.abs(got - float(keeps[d])) < 0.1)
                if not ok:
                    return False
    return True


def kernel(q, k, domain_label, time_label):
    global LAST_RESULT
    q = np.ascontiguousarray(np.asarray(q), dtype=np.float32)
    k = np.ascontiguousarray(np.asarray(k), dtype=np.float32)
    dl = np.asarray(domain_label).astype(np.int64)
    tl = np.asarray(time_label).astype(np.float64)

    perm = np.argsort(dl, kind="stable")
    counts = np.bincount(dl, minlength=ND).astype(np.int64)
    prefix = np.cumsum(counts) - counts
    keeps = np.floor(counts.astype(np.float64) * LAST_RATIO).astype(np.int64)

    eng = _engine_split(counts)
    use_safe = bool((counts < 16).any())
    mode = "safe" if use_safe else "fast"
    key = (tuple(counts.tolist()), tuple(keeps.tolist()), mode)
    if key not in _BUILD_CACHE:
        win, ni = (WIN_SAFE, NITER_SAFE) if use_safe else (WIN_FAST, NITER_FAST)
        _BUILD_CACHE[key] = _build(counts, prefix, keeps, eng, win, ni)
    nc = _BUILD_CACHE[key]

    kp = np.ascontiguousarray(k[perm])
    tlp = tl[perm].astype(np.float32)
    tb = np.ascontiguousarray(np.broadcast_to(tlp[None, :], (128, N)))
    ident = np.eye(128, dtype=np.float32)

    thr = np.zeros((128, RT * ND), np.float32)
    for r in range(RT):
        for d in range(ND):
            if eng[(r, d)] == "act":
                thr[:, r * ND + d] = float(2 * keeps[d] - counts[d])
            else:
                thr[:, r * ND + d] = float(keeps[d])

    in_maps = []
    for cid in range(NCORES):
        rows = slice(cid * RPC, (cid + 1) * RPC)
        tr = tl[rows]
        trlo = np.empty((128, RT), np.float32)
        trhi = np.empty((128, RT), np.float32)
        for r in range(RT):
            seg = tr[r * 128:(r + 1) * 128]
            trlo[:, r] = (seg - 49.5).astype(np.float32)
            trhi[:, r] = (seg + 49.5).astype(np.float32)
        in_maps.append({
            "qs": np.ascontiguousarray(q[rows]),
            "ko": np.ascontiguousarray(k[rows]),
            "kp": kp,
            "tb": tb,
            "trlo": trlo,
            "trhi": trhi,
            "thr": thr,
            "ident": ident,
        })

    LAST_RESULT = run_bass_kernel_spmd(nc, in_maps, core_ids=list(range(NCORES)))
    outs = LAST_RESULT.results

    if not use_safe and not _verify_counts(outs, counts, keeps, eng):
        # tight-window assumption violated: rerun with the safe build
        key = (tuple(counts.tolist()), tuple(keeps.tolist()), "safe")
        if key not in _BUILD_CACHE:
            _BUILD_CACHE[key] = _build(counts, prefix, keeps, eng,
                                       WIN_SAFE, NITER_SAFE)
        nc = _BUILD_CACHE[key]
        LAST_RESULT = run_bass_kernel_spmd(nc, in_maps,
                                           core_ids=list(range(NCORES)))
        outs = LAST_RESULT.results

    logits = np.empty((N, N + 1), np.float32)
    feature = np.empty((N, N), np.float32)
    inv_scatter = np.empty((RPC, N), np.float32)
    for cid in range(NCORES):
        rows = slice(cid * RPC, (cid + 1) * RPC)
        om = outs[cid]
        logits[rows, 0] = om["lpos"].T.reshape(RPC)
        inv_scatter[:, perm] = om["lneg"]
        logits[rows, 1:] = inv_scatter
        inv_scatter[:, perm] = om["feat"]
        feature[rows, :] = inv_scatter
    targets = np.zeros((N,), np.int32)
    return logits, targets, feature


# revision 14
# speedup vs baseline: 36326.7990x; 36326.7990x over previous
"""Trainium2 Bass kernel for ContrastSense-without-queue (topk_masking).

Strategy (8 NeuronCores, data-parallel over rows of q):
  - Host: stable-sort columns (k rows) by domain so each domain is a
    contiguous column segment; replicate k/time_label to all cores.
  - Device per core (512 q-rows): normalize q/k (fp32, Newton-refined
    rsqrt), compute sims = qn @ kn_perm.T with true-fp32 PE matmuls,
    then find, for every (row, domain), the exact k_d-th-largest
    similarity via a 25-step vectorized bisection (strict counts done
    with fused compare+accumulate ops on the Scalar and Vector
    engines), build the |dt|<50 time mask, and emit
    logits = -inf where masked else sims/temp.
  - Host: scatter columns back to the original order, prepend l_pos.

Outputs match the reference tuple (logits [N,N+1] f32, targets [N] i32,
feature [N,N] f32).
"""

import os
import sys

import numpy as np

for _p in ("/opt/trn_rl_repo", os.path.expanduser("~/.axon_site/_ro/trn_rl_repo")):
    if os.path.isdir(_p) and _p not in sys.path:
        sys.path.insert(0, _p)

import concourse.bass as bass  # noqa: E402
import concourse.mybir as mybir  # noqa: E402
from concourse import bacc  # noqa: E402
from concourse.bass_utils import run_bass_kernel_spmd  # noqa: E402
from concourse.tile import TileContext  # noqa: E402

F32 = mybir.dt.float32
OP = mybir.AluOpType
AF = mybir.ActivationFunctionType

N = 4096
C = 256
ND = 8
NCORES = 8
RPC = N // NCORES   # rows per core = 512
RT = RPC // 128     # row tiles per core = 4
LAST_RATIO = 0.7
TEMP = 0.07
TINV = float(1.0 / np.float32(TEMP))  # logits = v * (1/temp); <=1ulp vs v/temp
NEGBIG = -3.4028235e38  # most-negative normal fp32; doubled -> -inf

NITER_SAFE = 25
WIN_SAFE = (-1.002, 1.002)  # fully data-independent window
NITER_FAST = 21
# thresholds are 30th-pctile order stats of ~N(0, 1/256) samples: concentrated
# around -0.033 +- ~0.004; +-7 sigma window, device-verified with fallback
WIN_FAST = (-0.061, -0.005)

_BUILD_CACHE = {}
LAST_RESULT = None  # BassKernelResults of the most recent run (for test.py)


def _engine_split(counts):
    """Assign each (rowtile, domain) counting stream to ACT or DVE,
    greedily balancing modeled per-iteration cost."""
    act_cost = 0.0
    dve_cost = 1100.0  # DVE also runs the bracket-update ops each iter
    eng = {}
    pairs = [(r, d) for r in range(RT) for d in range(ND)]
    # big segments first for better greedy balance
    pairs.sort(key=lambda p: -counts[p[1]])
    for r, d in pairs:
        c = float(counts[d])
        a = 429.0 + 0.833 * c  # ACT sign+accum (sim-calibrated)
        v = 60.0 + 0.52 * c    # DVE ts+accum runs 2x (sim-calibrated)
        if act_cost + a <= dve_cost + v:
            eng[(r, d)] = "act"
            act_cost += a
        else:
            eng[(r, d)] = "dve"
            dve_cost += v
    return eng


def _build(counts, prefix, keeps, eng, win, niter):
    nc = bacc.Bacc("TRN2")
    cmax = int(max(int(c) for c in counts if c > 0))

    qs = nc.dram_tensor("qs", [RPC, C], F32, kind="ExternalInput")
    ko = nc.dram_tensor("ko", [RPC, C], F32, kind="ExternalInput")
    kp = nc.dram_tensor("kp", [N, C], F32, kind="ExternalInput")
    tb_d = nc.dram_tensor("tb", [128, N], F32, kind="ExternalInput")
    trlo_d = nc.dram_tensor("trlo", [128, RT], F32, kind="ExternalInput")
    trhi_d = nc.dram_tensor("trhi", [128, RT], F32, kind="ExternalInput")
    thr_d = nc.dram_tensor("thr", [128, RT * ND], F32, kind="ExternalInput")
    id_d = nc.dram_tensor("ident", [128, 128], F32, kind="ExternalInput")

    feat = nc.dram_tensor("feat", [RPC, N], F32, kind="ExternalOutput")
    lneg = nc.dram_tensor("lneg", [RPC, N], F32, kind="ExternalOutput")
    lpos = nc.dram_tensor("lpos", [128, RT], F32, kind="ExternalOutput")
    cchk = nc.dram_tensor("cchk", [128, RT * ND], F32, kind="ExternalOutput")

    NQ = RT          # q tiles
    NK = N // 128    # kp tiles = 32
    NCC = C // 128   # contraction chunks = 2
    NNC = N // 512   # 512-wide column chunks = 8

    with TileContext(nc) as tc:
        with tc.tile_pool(name="persist", bufs=1) as pp, \
             tc.tile_pool(name="vpool", bufs=1) as vp:

            tb = pp.tile([128, N], F32, tag="tb")
            nc.sync.dma_start(tb[:], tb_d[:])
            trlo = pp.tile([128, RT], F32, tag="trlo")
            nc.sync.dma_start(trlo[:], trlo_d[:])
            trhi = pp.tile([128, RT], F32, tag="trhi")
            nc.sync.dma_start(trhi[:], trhi_d[:])
            thr = pp.tile([128, RT * ND], F32, tag="thr")
            nc.sync.dma_start(thr[:], thr_d[:])

            v = [vp.tile([128, N], F32, tag=f"v{r}", name=f"v{r}") for r in range(RT)]

            # ---------------- load + normalize + transpose + matmul ----
            with tc.tile_pool(name="raw", bufs=1) as rp, \
                 tc.tile_pool(name="tpool", bufs=1) as tp, \
                 tc.tile_pool(name="pmm", bufs=4, space="PSUM") as pmm, \
                 tc.tile_pool(name="ptr", bufs=4, space="PSUM") as ptr:

                ident = tp.tile([128, 128], F32, tag="ident")
                nc.sync.dma_start(ident[:], id_d[:])

                nt = NQ + NQ + NK  # 40 norm streams: q, ko, kp
                nrm = pp.tile([128, nt], F32, tag="nrm")
                s_ssq = nrm  # alias names for clarity
                qraw = [rp.tile([128, C], F32, tag=f"q{r}", name=f"q{r}") for r in range(NQ)]
                koraw = [rp.tile([128, C], F32, tag=f"o{r}", name=f"o{r}") for r in range(NQ)]
                kraw = [rp.tile([128, C], F32, tag=f"k{j}", name=f"k{j}") for j in range(NK)]
                sq_scr = [rp.tile([128, C], F32, tag=f"sqs{i}", name=f"sqs{i}") for i in range(3)]

                for r in range(NQ):
                    nc.sync.dma_start(qraw[r][:], qs[r * 128:(r + 1) * 128, :])
                for r in range(NQ):
                    nc.sync.dma_start(koraw[r][:], ko[r * 128:(r + 1) * 128, :])
                for j in range(NK):
                    nc.sync.dma_start(kraw[j][:], kp[j * 128:(j + 1) * 128, :])

                allt = qraw + koraw + kraw
                for i, t in enumerate(allt):
                    nc.scalar.activation(sq_scr[i % 3][:], t[:], AF.Square,
                                         bias=0.0, scale=1.0,
                                         accum_out=s_ssq[:, i:i + 1])
                # rsqrt with one Newton step:  y1 = y0*(1.5 - 0.5*ssq*y0^2)
                sroot = pp.tile([128, nt], F32, tag="sroot")
                nc.scalar.activation(sroot[:], nrm[:], AF.Sqrt, bias=0.0, scale=1.0)
                y0 = pp.tile([128, nt], F32, tag="y0")
                nc.vector.reciprocal(y0[:], sroot[:])
                tmp = pp.tile([128, nt], F32, tag="nrmtmp")
                nc.vector.tensor_tensor(tmp[:], y0[:], y0[:], op=OP.mult)
                nc.vector.tensor_tensor(tmp[:], tmp[:], nrm[:], op=OP.mult)
                nc.vector.tensor_scalar(tmp[:], tmp[:], -0.5, 1.5, op0=OP.mult, op1=OP.add)
                nc.vector.tensor_tensor(y0[:], y0[:], tmp[:], op=OP.mult)

                # normalize in place
                for i, t in enumerate(allt):
                    nc.vector.tensor_scalar(t[:], t[:], y0[:, i:i + 1], None, op0=OP.mult)

                # l_pos = rowsum(qn*kon) * (1/temp)
                lpt = pp.tile([128, RT], F32, tag="lpt")
                for r in range(NQ):
                    nc.vector.tensor_tensor(sq_scr[r % 3][:], qraw[r][:], koraw[r][:], op=OP.mult)
                    nc.vector.tensor_reduce(lpt[:, r:r + 1], sq_scr[r % 3][:],
                                            axis=mybir.AxisListType.X, op=OP.add)
                nc.vector.tensor_scalar(lpt[:], lpt[:], TINV, None, op0=OP.mult)
                nc.sync.dma_start(lpos[:], lpt[:])

                # transposes: qnT [2 x (128 x 512)], knT [2 x (128 x 4096)]
                qnT = [tp.tile([128, RPC], F32, tag=f"qnT{cc}", name=f"qnT{cc}") for cc in range(NCC)]
                knT = [tp.tile([128, N], F32, tag=f"knT{cc}", name=f"knT{cc}") for cc in range(NCC)]
                # evacuate transposes with DVE only: keeps every matmul's
                # dependencies on a single engine clock (fp32 matmuls are
                # self-loading, so multi-sem waits can't be moved to LDW)
                for cc in range(NCC):
                    pt = ptr.tile([128, 512], F32, tag="ptr")
                    for r in range(NQ):
                        nc.tensor.transpose(pt[:, r * 128:(r + 1) * 128],
                                            qraw[r][:, cc * 128:(cc + 1) * 128], ident[:])
                    nc.vector.tensor_copy(qnT[cc][:, :], pt[:])
                for jg in range(NK // 4):
                    for cc in range(NCC):
                        pt = ptr.tile([128, 512], F32, tag="ptr")
                        for jj in range(4):
                            j = jg * 4 + jj
                            nc.tensor.transpose(pt[:, jj * 128:(jj + 1) * 128],
                                                kraw[j][:, cc * 128:(cc + 1) * 128], ident[:])
                        nc.vector.tensor_copy(knT[cc][:, jg * 512:(jg + 1) * 512], pt[:])

                # main matmul: v[r][:, n*512:(n+1)*512] = qnT.T @ knT chunks
                for r in range(RT):
                    for n in range(NNC):
                        pm = pmm.tile([128, 512], F32, tag="pmm")
                        for cc in range(NCC):
                            nc.tensor.matmul(pm[:],
                                             qnT[cc][:, r * 128:(r + 1) * 128],
                                             knT[cc][:, n * 512:(n + 1) * 512],
                                             start=(cc == 0), stop=(cc == NCC - 1))
                        nc.vector.tensor_copy(v[r][:, n * 512:(n + 1) * 512], pm[:])
                        nc.sync.dma_start(
                            feat[r * 128:(r + 1) * 128, n * 512:(n + 1) * 512],
                            v[r][:, n * 512:(n + 1) * 512])

            # ---------------- selection: vectorized bisection ----------
            NP = RT * ND
            lo = pp.tile([128, NP], F32, tag="lo")
            hi = pp.tile([128, NP], F32, tag="hi")
            mid = pp.tile([128, NP], F32, tag="mid")
            nmid = pp.tile([128, NP], F32, tag="nmid")
            cnt = pp.tile([128, NP], F32, tag="cnt")
            dec = pp.tile([128, NP], F32, tag="dec")
            st0 = pp.tile([128, NP], F32, tag="st0")
            su0 = pp.tile([128, NP], F32, tag="su0")
            su1 = pp.tile([128, NP], F32, tag="su1")
            wlo, whi = win
            mid0 = 0.5 * (wlo + whi)
            nc.vector.memset(lo[:], wlo)
            nc.vector.memset(hi[:], whi)
            nc.vector.memset(mid[:], mid0)
            nc.vector.memset(nmid[:], -mid0)

            with tc.tile_pool(name="cscr", bufs=3) as cs:
                for it in range(niter):
                    for r in range(RT):
                        for d in range(ND):
                            cd = int(counts[d])
                            if cd == 0:
                                continue
                            col = r * ND + d
                            s0 = int(prefix[d])
                            seg = v[r][:, s0:s0 + cd]
                            if eng[(r, d)] == "act":
                                o = cs.tile([128, cmax], F32, tag="sa")
                                nc.scalar.activation(o[:, :cd], seg, AF.Sign,
                                                     bias=nmid[:, col:col + 1],
                                                     scale=1.0,
                                                     accum_out=cnt[:, col:col + 1])
                            else:
                                o = cs.tile([128, cmax], F32, tag="sd")
                                nc.vector.tensor_scalar(o[:, :cd], seg,
                                                        mid[:, col:col + 1], None,
                                                        op0=OP.is_gt, op1=OP.add,
                                                        accum_out=cnt[:, col:col + 1])
                    nc.vector.tensor_tensor(dec[:], cnt[:], thr[:], op=OP.is_ge)
                    nc.vector.tensor_tensor(st0[:], mid[:], lo[:], op=OP.subtract)
                    nc.vector.tensor_tensor(st0[:], dec[:], st0[:], op=OP.mult)
                    nc.vector.tensor_tensor(lo[:], lo[:], st0[:], op=OP.add)
                    nc.vector.tensor_tensor(su0[:], mid[:], hi[:], op=OP.subtract)
                    nc.vector.tensor_tensor(su1[:], dec[:], su0[:], op=OP.mult)
                    nc.vector.tensor_tensor(su0[:], su0[:], su1[:], op=OP.subtract)
                    nc.vector.tensor_tensor(hi[:], hi[:], su0[:], op=OP.add)
                    if it < niter - 1:
                        nc.vector.tensor_tensor(mid[:], lo[:], hi[:], op=OP.add)
                        nc.vector.tensor_scalar(mid[:], mid[:], 0.5, None, op0=OP.mult)
                        nc.vector.tensor_scalar(nmid[:], mid[:], -1.0, None, op0=OP.mult)

            # verification: recount at tau = lo; host asserts exact counts
            nlo = pp.tile([128, NP], F32, tag="nlo")
            nc.vector.tensor_scalar(nlo[:], lo[:], -1.0, None, op0=OP.mult)
            with tc.tile_pool(name="vscr", bufs=3) as vs_p:
                for r in range(RT):
                    for d in range(ND):
                        cd = int(counts[d])
                        if cd == 0:
                            continue
                        col = r * ND + d
                        s0 = int(prefix[d])
                        seg = v[r][:, s0:s0 + cd]
                        if eng[(r, d)] == "act":
                            o = vs_p.tile([128, cmax], F32, tag="va", name="va")
                            nc.scalar.activation(o[:, :cd], seg, AF.Sign,
                                                 bias=nlo[:, col:col + 1],
                                                 scale=1.0,
                                                 accum_out=cnt[:, col:col + 1])
                        else:
                            o = vs_p.tile([128, cmax], F32, tag="vd", name="vd")
                            nc.vector.tensor_scalar(o[:, :cd], seg,
                                                    lo[:, col:col + 1], None,
                                                    op0=OP.is_gt, op1=OP.add,
                                                    accum_out=cnt[:, col:col + 1])
            nc.sync.dma_start(cchk[:], cnt[:])

            # ---------------- time mask + apply + store ----------------
            with tc.tile_pool(name="mke", bufs=1) as mk, \
                 tc.tile_pool(name="c2p", bufs=1) as c2p, \
                 tc.tile_pool(name="e2p", bufs=1) as e2p:
                for r in range(RT):
                    # time mask: m = (t_j > t_i - 49.5) && (t_j < t_i + 49.5)
                    c2 = c2p.tile([128, N], F32, tag="c2")
                    nc.gpsimd.tensor_scalar(c2[:], tb[:], trlo[:, r:r + 1], None,
                                            op0=OP.is_gt)
                    m = mk.tile([128, N], F32, tag="m")
                    nc.vector.scalar_tensor_tensor(m[:], tb[:], trhi[:, r:r + 1],
                                                   c2[:], op0=OP.is_lt,
                                                   op1=OP.logical_and)
                    # eliminate = (v <= lo) per domain segment
                    e2 = e2p.tile([128, N], F32, tag="e2")
                    for d in range(ND):
                        cd = int(counts[d])
                        if cd == 0:
                            continue
                        col = r * ND + d
                        s0 = int(prefix[d])
                        nc.vector.tensor_scalar(e2[:, s0:s0 + cd],
                                                v[r][:, s0:s0 + cd],
                                                lo[:, col:col + 1], None,
                                                op0=OP.is_le)
                    nc.vector.tensor_tensor(e2[:], e2[:], m[:], op=OP.logical_or)
                    # vs = v/temp (into c2, dead); t1 = e2*NEGBIG (into m, dead)
                    # logits = t1*2 + vs  (2*NEGBIG overflows to -inf)
                    nc.gpsimd.tensor_scalar(m[:], e2[:], NEGBIG, None, op0=OP.mult)
                    nc.scalar.mul(c2[:], v[r][:], TINV)
                    nc.vector.scalar_tensor_tensor(v[r][:], m[:], 2.0, c2[:],
                                                   op0=OP.mult, op1=OP.add)
                    nc.sync.dma_start(lneg[r * 128:(r + 1) * 128, :], v[r][:])

    nc.finalize()
    return nc


def _verify_counts(outs, counts, keeps, eng):
    for om in outs:
        cc = om["cchk"]
        for r in range(RT):
            for d in range(ND):
                if counts[d] == 0:
                    continue
                col = r * ND + d
                got = cc[:, col]
                if eng[(r, d)] == "act":
                    base = float(2 * keeps[d] - counts[d])
                    ok = np.all((got >= base - 0.1) & (got <= base + 1.1))
                else:
                    ok = np.all(np.abs(got - float(keeps[d])) < 0.1)
                if not ok:
                    return False
    return True


def kernel(q, k, domain_label, time_label):
    global LAST_RESULT
    q = np.ascontiguousarray(np.asarray(q), dtype=np.float32)
    k = np.ascontiguousarray(np.asarray(k), dtype=np.float32)
    dl = np.asarray(domain_label).astype(np.int64)
    tl = np.asarray(time_label).astype(np.float64)

    perm = np.argsort(dl, kind="stable")
    counts = np.bincount(dl, minlength=ND).astype(np.int64)
    prefix = np.cumsum(counts) - counts
    keeps = np.floor(counts.astype(np.float64) * LAST_RATIO).astype(np.int64)

    eng = _engine_split(counts)
    use_safe = bool((counts < 16).any())
    mode = "safe" if use_safe else "fast"
    key = (tuple(counts.tolist()), tuple(keeps.tolist()), mode)
    if key not in _BUILD_CACHE:
        win, ni = (WIN_SAFE, NITER_SAFE) if use_safe else (WIN_FAST, NITER_FAST)
        _BUILD_CACHE[key] = _build(counts, prefix, keeps, eng, win, ni)
    nc = _BUILD_CACHE[key]

    kp = np.ascontiguousarray(k[perm])
    tlp = tl[perm].astype(np.float32)
    tb = np.ascontiguousarray(np.broadcast_to(tlp[None, :], (128, N)))
    ident = np.eye(128, dtype=np.float32)

    thr = np.zeros((128, RT * ND), np.float32)
    for r in range(RT):
        for d in range(ND):
            if eng[(r, d)] == "act":
                thr[:, r * ND + d] = float(2 * keeps[d] - counts[d])
            else:
                thr[:, r * ND + d] = float(keeps[d])

    in_maps = []
    for cid in range(NCORES):
        rows = slice(cid * RPC, (cid + 1) * RPC)
        tr = tl[rows]
        trlo = np.empty((128, RT), np.float32)
        trhi = np.empty((128, RT), np.float32)
        for r in range(RT):
            seg = tr[r * 128:(r + 1) * 128]
            trlo[:, r] = (seg - 49.5).astype(np.float32)
            trhi[:, r] = (seg + 49.5).astype(np.float32)
        in_maps.append({
            "qs": np.ascontiguousarray(q[rows]),
            "ko": np.ascontiguousarray(k[rows]),
            "kp": kp,
            "tb": tb,
            "trlo": trlo,
            "trhi": trhi,
            "thr": thr,
            "ident": ident,
        })

    LAST_RESULT = run_bass_kernel_spmd(nc, in_maps, core_ids=list(range(NCORES)))
    outs = LAST_RESULT.results

    if not use_safe and not _verify_counts(outs, counts, keeps, eng):
        # tight-window assumption violated: rerun with the safe build
        key = (tuple(counts.tolist()), tuple(keeps.tolist()), "safe")
        if key not in _BUILD_CACHE:
            _BUILD_CACHE[key] = _build(counts, prefix, keeps, eng,
                                       WIN_SAFE, NITER_SAFE)
        nc = _BUILD_CACHE[key]
        LAST_RESULT = run_bass_kernel_spmd(nc, in_maps,
                                           core_ids=list(range(NCORES)))
        outs = LAST_RESULT.results

    logits = np.empty((N, N + 1), np.float32)
    feature = np.empty((N, N), np.float32)
    inv_scatter = np.empty((RPC, N), np.float32)
    for cid in range(NCORES):
        rows = slice(cid * RPC, (cid + 1) * RPC)
        om = outs[cid]
        logits[rows, 0] = om["lpos"].T.reshape(RPC)
        inv_scatter[:, perm] = om["lneg"]
        logits[rows, 1:] = inv_scatter
        inv_scatter[:, perm] = om["feat"]
        feature[rows, :] = inv_scatter
    targets = np.zeros((N,), np.int32)
    return logits, targets, feature
